# revision 1
# baseline (speedup 1.0000x reference)
"""Trainium2 Bass kernel for CapsNet dynamic routing (nn_Capsule_34342558498916).

Full inputs:  u_vecs (64, 64, 1024) f32, W (1024, 32, 64, 16) f32
Full output:  (64, 16, 32) f32  == transpose(v, (0, 2, 1)) of v (B, N, D)

Sharding: capsule dim N=32 split across 8 cores (4 capsules each).  Every core
sees all of u_vecs and its W[:, n_l] slice; the routing loop (softmax over
the full u axis) is then entirely core-local, so no collectives are needed.

Per-core design:
  u_hat SBUF [128=(par,b), (t, d, n4)] f32, u = 2t+par  (par packs u-parity
  into the two 64-partition halves; the phase-1 einsum matmuls run in the
  two disjoint PE quadrants via base_partition-derived tile_position).
  Inputs stay f32: logits b = u_hat . v are amplified ~40x through exp, so
  fp16 inputs/u_hat alone cost ~1.5e-2 final error; f32 keeps it ~1e-3.
  Routing contractions run on the PE as accumulating identity/fold matmuls
  (the s~ = sum_u e*u_hat pass uses an 8-slot PSUM accumulator: 64 matmuls
  of N=512 instead of 512 of N=64, slots folded by one DVE reduce; its fp16
  product tensor halves PE stream cost).  Elementwise multiplies run on DVE
  (f32, 1x) with 1/3 offloaded to GpSimd (no port conflict: f32 1x ops do
  not use DVE's shared second port).  exp on ACT with fused per-partition
  bias (= -rowmax), logits kept f32 in SBUF; b-updates accumulate in
  transient PSUM then fold into SBUF f32 (cross-iteration PSUM accumulation
  is unreliable).  Local softmax max-subtraction is exact (u axis is fully
  core-local), avoiding the f32 exp overflow that |logits| > 88 causes.
"""

import os
import sys

import numpy as np

for _p in ("/opt/trn_rl_repo", "/opt/pypackages"):
    if _p not in sys.path:
        sys.path.append(_p)

import concourse.bass as bass
from concourse import bacc
import concourse.mybir as mybir
from concourse import tile
from concourse.bass_utils import run_bass_kernel_spmd

# Problem dims (hardcoded per harness contract)
B, C, U, N, D = 64, 64, 1024, 32, 16
NCORES = 8
NL = N // NCORES          # 4 capsules per core
T = U // 2                # 512 u-pairs
DN = D * NL               # 64 = matmul free dim (d, n4)
P = 128
EPS = 1e-8
ROUTINGS = 3

dt = mybir.dt
AF = mybir.ActivationFunctionType
ALU = mybir.AluOpType

_COMPILED = {}
DEBUG = False
SPLIT_POOL = True


def _squash_core(nc, sm, s_red_ap, zi_or_scale, s_sb, v16_dst):
    """s_red_ap: un-normalized s~ [B, D, NL]; s = s~ * zi or * scalar;
    v = squash(s).  Returns v_sb."""
    if isinstance(zi_or_scale, float):
        nc.vector.tensor_scalar_mul(s_sb[:], s_red_ap, zi_or_scale)
    else:
        zi_bc = zi_or_scale[:].unsqueeze(1).broadcast_to([B, D, NL])
        nc.vector.tensor_mul(s_sb[:], s_red_ap, zi_bc)
    ssq = sm.tile([B, D, NL], dt.float32, tag="ssq")
    nc.vector.tensor_mul(ssq[:], s_sb[:], s_sb[:])
    s2 = sm.tile([B, NL], dt.float32, tag="s2")
    # sum over d (innermost after free transpose)
    nc.vector.tensor_reduce(
        s2[:], ssq[:].transpose([0, 2, 1]), axis=mybir.AxisListType.X, op=ALU.add
    )
    s2e = sm.tile([B, NL], dt.float32, tag="s2e")
    nc.vector.tensor_scalar_add(s2e[:], s2[:], EPS)
    rt = sm.tile([B, NL], dt.float32, tag="rt")
    nc.scalar.activation(rt[:], s2e[:], AF.Sqrt)
    den = sm.tile([B, NL], dt.float32, tag="den")
    nc.vector.tensor_scalar_add(den[:], s2e[:], 1.0)
    deni = sm.tile([B, NL], dt.float32, tag="deni")
    nc.vector.reciprocal(deni[:], den[:])
    f = sm.tile([B, NL], dt.float32, tag="f")
    nc.vector.tensor_mul(f[:], rt[:], deni[:])
    v_sb = sm.tile([B, D, NL], dt.float32, tag="v_sb")
    nc.vector.tensor_mul(v_sb[:], s_sb[:], f[:].unsqueeze(1).broadcast_to([B, D, NL]))
    nc.vector.tensor_copy(v16_dst, v_sb[:])
    return v_sb


def _squash_and_v(nc, sm, s_ps8, zi_or_scale, s_sb, v16_dst):
    """Slotted PSUM accumulator [B, 8, D, NL] -> fold slots -> squash."""
    s_red = sm.tile([B, D, NL], dt.float32, tag="s_red")
    nc.vector.tensor_reduce(
        s_red[:], s_ps8[:].transpose([0, 2, 3, 1]),
        axis=mybir.AxisListType.X, op=ALU.add,
    )
    return _squash_core(nc, sm, s_red[:], zi_or_scale, s_sb, v16_dst)


def _build_program():
    nc = bacc.Bacc()

    ut = nc.dram_tensor("ut", [P, T, B], dt.float32, kind="ExternalInput")
    wt = nc.dram_tensor("wt", [P, T, DN], dt.float32, kind="ExternalInput")
    ident_d = nc.dram_tensor("ident", [P, P], dt.float32, kind="ExternalInput")
    fold_d = nc.dram_tensor("fold", [P, B], dt.float32, kind="ExternalInput")
    fold16_d = nc.dram_tensor("fold16", [P, B], dt.float16, kind="ExternalInput")
    out_d = nc.dram_tensor("out", [B, D, NL], dt.float32, kind="ExternalOutput")
    if DEBUG:
        dbg_v1 = nc.dram_tensor("dbg_v1", [B, D, NL], dt.float32, kind="ExternalOutput")
        dbg_v2 = nc.dram_tensor("dbg_v2", [B, D, NL], dt.float32, kind="ExternalOutput")
        dbg_b = nc.dram_tensor("dbg_b", [P, 128, NL], dt.float32, kind="ExternalOutput")
        dbg_e = nc.dram_tensor("dbg_e", [P, T, NL], dt.float32, kind="ExternalOutput")
        dbg_s = nc.dram_tensor("dbg_s", [B, D, NL], dt.float32, kind="ExternalOutput")
        dbg_vb = nc.dram_tensor("dbg_vb", [P, D, NL], dt.float32, kind="ExternalOutput")

    with tile.TileContext(nc) as tc:
        with (
            tc.tile_pool(name="big", bufs=1) as big,
            tc.tile_pool(name="wts", bufs=3) as wts,
            tc.tile_pool(name="prod", bufs=3) as prodp,
            tc.tile_pool(name="sm", bufs=1) as sm,
            tc.tile_pool(name="psB", bufs=3, space="PSUM") as psB,
            tc.tile_pool(name="psS", bufs=1, space="PSUM") as psS,
        ):
            u_hat = big.tile([P, T, D, NL], dt.float32, tag="u_hat")
            e_sb = big.tile([P, T, NL], dt.float16, tag="e_sb")
            ident = sm.tile([P, P], dt.float32, tag="ident")
            fold = sm.tile([P, B], dt.float32, tag="fold")
            fold16 = sm.tile([P, B], dt.float16, tag="fold16")
            vbc = sm.tile([P, D, NL], dt.float32, tag="vbc")
            mneg = sm.tile([P, NL], dt.float32, tag="mneg")
            bmax = sm.tile([P, NL], dt.float32, tag="bmax")
            bmax_p = sm.tile([P, 4, NL], dt.float32, tag="bmax_p")
            tmp64 = sm.tile([B, NL], dt.float32, tag="tmp64")
            z_p = sm.tile([P, NL], dt.float32, tag="z_p")
            z_f = sm.tile([B, NL], dt.float32, tag="z_f")
            zi = sm.tile([B, NL], dt.float32, tag="zi")
            s_sb = sm.tile([B, D, NL], dt.float32, tag="s_sb")

            nc.sync.dma_start(ident[:], ident_d[:])
            nc.sync.dma_start(fold[:], fold_d[:])
            nc.sync.dma_start(fold16[:], fold16_d[:])

            # ---------------- Phase 1: u_hat = einsum over c ----------------
            # s~1 = sum_u u_hat: 4 DVE quarter-reduces, pipelined behind
            # the phase-1 evacuations (keeps PE free for the einsum matmuls).
            s1acc = sm.tile([P, 4, DN], dt.float32, tag="s1acc")
            TCH = 16   # t per DMA chunk
            with tc.tile_pool(name="pp", bufs=3, space="PSUM") as pp:
                for ch in range(T // TCH):
                    wt_ch = wts.tile([P, TCH, DN], dt.float32, tag="wt_ch")
                    nc.sync.dma_start(wt_ch[:], wt[:, ch * TCH:(ch + 1) * TCH, :])
                    ut_ch = wts.tile([P, TCH, B], dt.float32, tag="ut_ch")
                    nc.sync.dma_start(ut_ch[:], ut[:, ch * TCH:(ch + 1) * TCH, :])
                    for g in range(TCH // 8):
                        ps = pp.tile([P, 8, DN], dt.float32, tag="pp")
                        for j in range(8):
                            tl = g * 8 + j
                            t = ch * TCH + tl
                            nc.tensor.matmul(
                                ps[0:64, j, :], ut_ch[0:64, tl, :],
                                wt_ch[0:64, tl, :], start=True, stop=True,
                            )
                            nc.tensor.matmul(
                                ps[64:128, j, :], ut_ch[64:128, tl, :],
                                wt_ch[64:128, tl, :], start=True, stop=True,
                            )
                        t0 = ch * TCH + g * 8
                        dst = u_hat[:, t0:t0 + 8, :, :]
                        src = ps[:].rearrange("p e (d n) -> p e d n", d=D)
                        if g % 2 == 0:
                            nc.vector.tensor_copy(dst, src)
                        else:
                            nc.scalar.copy(dst, src)
                    if ch % 8 == 7:
                        q4 = ch // 8
                        nc.vector.tensor_reduce(
                            s1acc[:, q4, :],
                            u_hat[:, q4 * 128:(q4 + 1) * 128, :, :]
                                .transpose([0, 2, 3, 1]),
                            axis=mybir.AxisListType.X, op=ALU.add,
                        )

            # B-logits live in SBUF f32; per-iteration updates accumulate in
            # transient PSUM tiles then fold in via DVE (cross-iteration PSUM
            # accumulation is not reliable).
            b_sb = big.tile([P, T, NL], dt.float32, tag="b_sb")

            # ---------------- Iteration 1: uniform c -> v1 ----------------
            s1red = sm.tile([P, DN], dt.float32, tag="s1red")
            nc.vector.tensor_reduce(
                s1red[:], s1acc[:].transpose([0, 2, 1]),
                axis=mybir.AxisListType.X, op=ALU.add,
            )
            s1tmp = sm.tile([B, DN], dt.float32, tag="s1tmp")
            nc.sync.dma_start(s1tmp[:], s1red[64:128, :])
            s1f = sm.tile([B, DN], dt.float32, tag="s1f")
            nc.vector.tensor_add(s1f[:], s1red[0:64, :], s1tmp[:])
            v1_sb = _squash_core(
                nc, sm, s1f[:].rearrange("b (d n) -> b d n", d=D),
                1.0 / U, s_sb, vbc[0:64, :, :],
            )
            nc.sync.dma_start(vbc[64:128, :, :], vbc[0:64, :, :])
            if DEBUG:
                nc.sync.dma_start(dbg_v1[:], v1_sb[:])
                nc.sync.dma_start(dbg_s[:], s_sb[:])

            # ---------------- Iterations 2..3 ----------------
            for it in range(1, ROUTINGS):
                if DEBUG and it == 2:
                    nc.sync.dma_start(dbg_vb[:], vbc[:])
                # b += sum_d u_hat * v   (DVE mult + PE identity-accumulate)
                for mm in range(16):
                    tbase = mm * 32
                    pr = prodp.tile([P, 32, D, NL], dt.float32, tag="pr")
                    mul_eng = nc.gpsimd if (SPLIT_POOL and mm % 3 == 2) else nc.vector
                    mul_eng.tensor_mul(
                        pr[:], u_hat[:, tbase:tbase + 32, :, :],
                        vbc[:].unsqueeze(1).broadcast_to([P, 32, D, NL]),
                    )
                    dst = b_sb[:, tbase:tbase + 32, :]
                    if mm % 3 == 1:
                        # d-reduce on DVE (PE relief)
                        red = sm.tile([P, 32, NL], dt.float32, tag="bu_dve")
                        nc.vector.tensor_reduce(
                            red[:], pr[:].transpose([0, 1, 3, 2]),
                            axis=mybir.AxisListType.X, op=ALU.add,
                        )
                        if it == 1:
                            nc.vector.tensor_copy(dst, red[:])
                        else:
                            nc.vector.tensor_add(dst, dst, red[:])
                    else:
                        bu = psB.tile([P, 32, NL], dt.float32, tag="bu")
                        for d in range(D):
                            nc.tensor.matmul(
                                bu[:], ident[:], pr[:, :, d, :],
                                start=(d == 0), stop=(d == D - 1),
                            )
                        if it == 1:
                            nc.vector.tensor_copy(dst, bu[:])
                        else:
                            nc.vector.tensor_add(dst, dst, bu[:])
                    if mm % 4 == 3:
                        q4 = mm // 4
                        nc.vector.tensor_reduce(
                            bmax_p[:, q4, :],
                            b_sb[:, q4 * 128:(q4 + 1) * 128, :].transpose([0, 2, 1]),
                            axis=mybir.AxisListType.X, op=ALU.max,
                        )
                # row max over u (for exp stability)
                nc.vector.tensor_reduce(
                    bmax[:], bmax_p[:].transpose([0, 2, 1]),
                    axis=mybir.AxisListType.X, op=ALU.max,
                )
                nc.sync.dma_start(tmp64[:], bmax[64:128, :])
                nc.vector.tensor_tensor(bmax[0:64, :], bmax[0:64, :], tmp64[:], op=ALU.max)
                nc.vector.tensor_scalar_mul(mneg[0:64, :], bmax[0:64, :], -1.0)
                nc.sync.dma_start(mneg[64:128, :], mneg[0:64, :])

                # e = exp(b - rowmax)  (ACT with fused per-partition bias)
                for j in range(NL):
                    nc.scalar.activation(
                        e_sb[:, :, j], b_sb[:, :, j], AF.Exp,
                        bias=mneg[:, j:j + 1], scale=1.0,
                    )
                # Z = sum_u e
                nc.vector.tensor_reduce(
                    z_p[:], e_sb[:].transpose([0, 2, 1]),
                    axis=mybir.AxisListType.X, op=ALU.add,
                )
                nc.sync.dma_start(tmp64[:], z_p[64:128, :])
                nc.vector.tensor_tensor(z_f[:], z_p[0:64, :], tmp64[:], op=ALU.add)
                nc.vector.reciprocal(zi[:], z_f[:])

                # s~ = sum_u e * u_hat  (DVE mult + PE slotted fold-accumulate)
                s_ps = psS.tile([B, 8, D, NL], dt.float32, tag="s_ps")
                for m in range(16):
                    tbase = m * 32
                    pr2 = prodp.tile([P, 32, D, NL], dt.float16, tag="pr2")
                    mul_eng = nc.gpsimd if (SPLIT_POOL and m % 3 == 2) else nc.vector
                    mul_eng.tensor_mul(
                        pr2[:], u_hat[:, tbase:tbase + 32, :, :],
                        e_sb[:, tbase:tbase + 32, :].unsqueeze(2)
                            .broadcast_to([P, 32, D, NL]),
                    )
                    for g in range(4):
                        nc.tensor.matmul(
                            s_ps[:], fold16[:], pr2[:, g * 8:(g + 1) * 8, :, :],
                            start=(m == 0 and g == 0),
                            stop=(m == 15 and g == 3),
                        )
                v_sb = _squash_and_v(nc, sm, s_ps, zi, s_sb, vbc[0:64, :, :])
                if it < ROUTINGS - 1:
                    nc.sync.dma_start(vbc[64:128, :, :], vbc[0:64, :, :])
                if DEBUG and it == 2:
                    nc.sync.dma_start(dbg_v2[:], v_sb[:])
                    nc.sync.dma_start(dbg_b[:], b_sb[:, 0:128, :])
                    dbg_e16 = sm.tile([P, T, NL], dt.float32, tag="dbg_e16")
                    nc.vector.tensor_copy(dbg_e16[:], e_sb[:])
                    nc.sync.dma_start(dbg_e[:], dbg_e16[:])

            nc.sync.dma_start(out_d[:], v_sb[:])

    nc.finalize()
    return nc


def _prep_inputs(u_vecs, W):
    """Host-side shard + relayout.  Returns per-core input maps."""
    u32 = np.ascontiguousarray(u_vecs, dtype=np.float32)
    # [(par,c), t, b]:  u = 2t + par
    utc = u32.transpose(1, 2, 0).reshape(C, T, 2, B)           # c, t, par, b
    ut2 = np.ascontiguousarray(utc.transpose(2, 0, 1, 3)).reshape(P, T, B)
    ident = np.eye(P, dtype=np.float32)
    fold = np.tile(np.eye(B, dtype=np.float32), (2, 1))        # [128, 64]
    in_maps = []
    Wf = np.ascontiguousarray(W, dtype=np.float32)
    for k in range(NCORES):
        wk = Wf[:, k * NL:(k + 1) * NL]                        # [U, NL, C, D]
        # [(par,c), t, (d, n4)]
        wkt = wk.transpose(0, 2, 3, 1).reshape(T, 2, C, D * NL)  # t, par, c, dn
        wt2 = np.ascontiguousarray(wkt.transpose(1, 2, 0, 3)).reshape(P, T, DN)
        in_maps.append({"ut": ut2, "wt": wt2, "ident": ident, "fold": fold,
                        "fold16": fold.astype(np.float16)})
    return in_maps


def kernel(u_vecs: np.ndarray, W: np.ndarray) -> np.ndarray:
    if "nc" not in _COMPILED:
        _COMPILED["nc"] = _build_program()
    nc = _COMPILED["nc"]
    in_maps = _prep_inputs(np.asarray(u_vecs), np.asarray(W))
    res = run_bass_kernel_spmd(nc, in_maps, list(range(NCORES)))
    outs = [np.asarray(res.results[k]["out"]) for k in range(NCORES)]
    return np.concatenate(outs, axis=-1).astype(np.float32)  # (B, D, N)

